# revision 5
# baseline (speedup 1.0000x reference)
"""Causal GQA attention for Trainium2, sharded across 8 NeuronCores.

Problem: q [2, 2048, 32, 128], k/v [2, 2048, 8, 128] fp32, causal,
GQA group = 4. Sharding: core i gets kv-head i plus its 4 q-heads
(heads 4i..4i+3), both batch elements. Each core runs the same program
(SPMD) on its shard; outputs are concatenated on the head axis.

Per-core kernel (flash-attention style, transposed-S layout):
  S^T[k, q] = (Q K^T)^T computed as lhsT=K^T-chunk [d,128], rhs=Q^T [d,512].
  exp on ScalarE (scale folded in) writes P^T [k, q] straight into the
  layout the PV matmul wants as its moving operand; V needs no transpose.
  out^T[d, q] accumulates over k-chunks in PSUM; denominator accumulates
  via a ones[128,1] matmul. Causal masking = column-restricted matmuls +
  one [128,128] mask add on diagonal blocks.
"""

import math

import numpy as np

import concourse.bass as bass
import concourse.tile as tile
from concourse import bacc, mybir
from concourse.bass_utils import run_bass_kernel_spmd
from concourse.masks import make_identity

P = 128
F32 = mybir.dt.float32

# Matmul operand dtype: float32 is exact (4 cyc/row), float32r is a
# TF32-like rounded mode (1 cyc/row at N>=256).
MM_DT = mybir.dt.float32
# Normalization path: "gpsimd_bcast" uses the GpSimd partition-broadcast
# custom instruction; "pe_bcast" broadcasts recip(den) across partitions
# via a PE outer product instead.
NORM_MODE = "gpsimd_bcast"
MASK_VAL = -1e6


def emit_attention(nc, tc, ctx, q_ap, k_ap, v_ap, o_ap, B, QL, KL, HL, D,
                   dt_mm=None):
    """Emit the per-core attention program.

    q_ap/o_ap: [B, QL, HL, D] dram APs; k_ap/v_ap: [B, KL, D].
    """
    assert D == P
    dt_mm = dt_mm or MM_DT
    QT = 512                       # q tile (psum bank = 512 fp32)
    KC = P                         # k chunk (contraction granularity)
    n_qt = QL // QT
    n_kc_total = KL // KC
    qt_per_kc = QT // KC           # k-chunks per q-tile (diag straddle count)
    scale = 1.0 / math.sqrt(D)

    sb = ctx.enter_context(tc.tile_pool(name="sb", bufs=1))
    sb_q = ctx.enter_context(tc.tile_pool(name="sb_q", bufs=2))
    sb_pt = ctx.enter_context(tc.tile_pool(name="sb_pt", bufs=4))
    sb_o = ctx.enter_context(tc.tile_pool(name="sb_o", bufs=2))
    ps_s = ctx.enter_context(tc.tile_pool(name="ps_s", bufs=3, space="PSUM"))
    ps_o = ctx.enter_context(tc.tile_pool(name="ps_o", bufs=2, space="PSUM"))
    ps_t = ctx.enter_context(tc.tile_pool(name="ps_t", bufs=2, space="PSUM"))
    ps_d = ctx.enter_context(tc.tile_pool(name="ps_d", bufs=1, space="PSUM"))

    # --- constants ---
    identity = sb.tile([P, P], F32, name="identity")
    make_identity(nc, identity[:])
    # causal mask in S^T coords: [k_local, q_local], fill where q < k
    mask = sb.tile([P, P], F32, name="mask")
    nc.gpsimd.memset(mask[:], 0.0)
    nc.gpsimd.affine_select(
        out=mask[:], in_=mask[:], compare_op=mybir.AluOpType.is_ge,
        fill=MASK_VAL, base=0, pattern=[[1, P]], channel_multiplier=-1,
    )
    ones_f32 = sb.tile([P, 1], F32, name="ones_f32")
    nc.gpsimd.memset(ones_f32[:], 1.0)
    ones = sb.tile([P, 1], dt_mm, name="ones")
    nc.vector.tensor_copy(ones[:], ones_f32[:])
    onesrow = sb.tile([1, P], F32, name="onesrow")
    nc.gpsimd.memset(onesrow[:], 1.0)

    # --- per-batch K^T and V, resident in SBUF ---
    KTs, Vs = [], []
    for b in range(B):
        kstage = sb.tile([P, n_kc_total, P], F32, name=f"kstage{b}")
        nc.sync.dma_start(
            kstage[:], k_ap[b].rearrange("(c p) d -> p c d", p=P))
        KT = sb.tile([P, KL], dt_mm, name=f"KT{b}")
        for g in range(0, n_kc_total, 4):
            tp = ps_t.tile([P, 512], F32, tag="tp")
            for j in range(4):
                nc.tensor.transpose(
                    tp[:, j * P:(j + 1) * P], kstage[:, g + j, :], identity[:])
            nc.vector.tensor_copy(KT[:, g * P:(g + 4) * P], tp[:])
        KTs.append(KT)

        vstage = sb.tile([P, n_kc_total, P], F32, name=f"vstage{b}")
        nc.sync.dma_start(
            vstage[:], v_ap[b].rearrange("(c p) d -> p c d", p=P))
        V = sb.tile([P, n_kc_total, P], dt_mm, name=f"V{b}")
        nc.vector.tensor_copy(V[:], vstage[:])
        Vs.append(V)

    # --- main loops ---
    for b in range(B):
        KT, V = KTs[b], Vs[b]
        for h in range(HL):
            for qt in range(n_qt):
                q0 = qt * QT
                n_kc = (q0 + QT) // KC

                # load + transpose Q^T tile [d, QT]
                qstage = sb_q.tile([P, qt_per_kc, P], F32, tag="qstage")
                nc.sync.dma_start(
                    qstage[:],
                    q_ap[b, q0:q0 + QT, h, :].rearrange(
                        "(j p) d -> p j d", p=P))
                qtp = ps_t.tile([P, 512], F32, tag="tp")
                for j in range(qt_per_kc):
                    nc.tensor.transpose(
                        qtp[:, j * P:(j + 1) * P], qstage[:, j, :],
                        identity[:])
                QTt = sb_q.tile([P, QT], dt_mm, tag="qtt")
                nc.vector.tensor_copy(QTt[:], qtp[:])

                O_ps = ps_o.tile([P, QT], F32, tag="o")
                D_ps = ps_d.tile([1, QT], F32, tag="d")

                # software-pipelined k-chunk loop: S(kc) runs 2 ahead of
                # O/D(kc) so PE never stalls on the ACT exp.
                pts = {}

                def emit_S(kc, qt=qt, b=b, QTt=QTt, pts=pts):
                    diag_j = kc - qt * qt_per_kc
                    col0 = max(0, diag_j) * KC if diag_j >= 0 else 0
                    ncols = QT - col0
                    S_ps = ps_s.tile([P, QT], F32, tag="s")
                    nc.tensor.matmul(
                        S_ps[:, col0:], KTs[b][:, kc * KC:(kc + 1) * KC],
                        QTt[:, col0:], start=True, stop=True,
                        skip_group_check=True)
                    if diag_j >= 0:
                        nc.vector.tensor_add(
                            S_ps[:, col0:col0 + P], S_ps[:, col0:col0 + P],
                            mask[:])
                    PT = sb_pt.tile([P, QT], dt_mm, tag="pt")
                    nc.scalar.activation(
                        PT[:, col0:], S_ps[:, col0:],
                        mybir.ActivationFunctionType.Exp, scale=scale)
                    pts[kc] = (PT, col0)

                def emit_OD(kc, n_kc=n_kc, O_ps=O_ps, D_ps=D_ps, b=b,
                            pts=pts):
                    PT, col0 = pts.pop(kc)
                    first, last = kc == 0, kc == n_kc - 1
                    nc.tensor.matmul(
                        O_ps[:, col0:], Vs[b][:, kc, :], PT[:, col0:],
                        start=first, stop=last, skip_group_check=True)
                    nc.tensor.matmul(
                        D_ps[:1, col0:], ones[:, :1], PT[:, col0:],
                        start=first, stop=last, skip_group_check=True)

                for kc in range(n_kc):
                    emit_S(kc)
                    if kc >= 2:
                        emit_OD(kc - 2)
                emit_OD(n_kc - 2)
                emit_OD(n_kc - 1)

                # epilogue: normalize, transpose back to [q, d], store
                den = sb_o.tile([1, QT], F32, tag="den")
                nc.vector.tensor_copy(den[:], D_ps[:1, :])
                nc.vector.reciprocal(den[:], den[:])
                O_sb = sb_o.tile([P, QT], F32, tag="osb")
                if NORM_MODE == "gpsimd_bcast":
                    bc_sb = sb_o.tile([P, QT], F32, tag="bcsb")
                    nc.gpsimd.partition_broadcast(bc_sb[:], den[:])
                    nc.vector.tensor_mul(O_sb[:], O_ps[:], bc_sb[:])
                else:
                    bc_ps = ps_t.tile([P, 512], F32, tag="tp")
                    nc.tensor.matmul(
                        bc_ps[:], onesrow[:], den[:], start=True, stop=True,
                        skip_group_check=True)
                    bc_sb = sb_o.tile([P, QT], F32, tag="bcsb")
                    nc.vector.tensor_copy(bc_sb[:], bc_ps[:])
                    nc.vector.tensor_mul(O_sb[:], O_ps[:], bc_sb[:])
                otp = ps_t.tile([P, 512], F32, tag="tp")
                for j in range(qt_per_kc):
                    nc.tensor.transpose(
                        otp[:, j * P:(j + 1) * P], O_sb[:, j * P:(j + 1) * P],
                        identity[:])
                Ot = sb_o.tile([P, QT], F32, tag="ot")
                nc.vector.tensor_copy(Ot[:], otp[:])
                nc.sync.dma_start(
                    o_ap[b, q0:q0 + QT, h, :].rearrange(
                        "(j p) d -> p j d", p=P),
                    Ot[:].rearrange("p (j d) -> p j d", d=P))


def build_nc(B=2, QL=2048, KL=2048, HL=4, D=128, dt_mm=None):
    nc = bacc.Bacc("TRN2", target_bir_lowering=False, debug=False,
                   num_devices=8)
    q = nc.dram_tensor("q", [B, QL, HL, D], F32, kind="ExternalInput")
    k = nc.dram_tensor("k", [B, KL, D], F32, kind="ExternalInput")
    v = nc.dram_tensor("v", [B, KL, D], F32, kind="ExternalInput")
    o = nc.dram_tensor("out", [B, QL, HL, D], F32, kind="ExternalOutput")
    from contextlib import ExitStack
    with tile.TileContext(nc) as tc:
        with ExitStack() as ctx:
            emit_attention(nc, tc, ctx, q.ap(), k.ap(), v.ap(), o.ap(),
                           B, QL, KL, HL, D, dt_mm=dt_mm)
    nc.compile()
    return nc


_NC_CACHE = {}


def kernel(q: np.ndarray, k: np.ndarray, v: np.ndarray) -> np.ndarray:
    B, QL, H, D = q.shape
    KL, KVH = k.shape[1], k.shape[2]
    n_cores = 8
    HL = H // n_cores            # q-heads per core (4)
    assert KVH == n_cores and H == 32 and D == 128

    if "nc" not in _NC_CACHE:
        _NC_CACHE["nc"] = build_nc(B=B, QL=QL, KL=KL, HL=HL, D=D)
    nc = _NC_CACHE["nc"]

    q = np.asarray(q, dtype=np.float32)
    k = np.asarray(k, dtype=np.float32)
    v = np.asarray(v, dtype=np.float32)
    in_maps = []
    for c in range(n_cores):
        in_maps.append({
            "q": np.ascontiguousarray(q[:, :, HL * c:HL * (c + 1), :]),
            "k": np.ascontiguousarray(k[:, :, c, :]),
            "v": np.ascontiguousarray(v[:, :, c, :]),
        })
    res = run_bass_kernel_spmd(nc, in_maps, list(range(n_cores)))
    return np.concatenate([r["out"] for r in res.results], axis=2)


# revision 17
# speedup vs baseline: 1.0993x; 1.0993x over previous
"""Causal GQA attention for Trainium2, sharded across 8 NeuronCores.

Problem: q [2, 2048, 32, 128], k/v [2, 2048, 8, 128] fp32, causal,
GQA group = 4. Sharding: core i gets kv-head i plus its 4 q-heads
(heads 4i..4i+3), both batch elements. Each core runs the same program
(SPMD) on its shard; outputs are concatenated on the head axis.

Per-core kernel (flash-attention style, transposed-S layout):
  S^T[k, q] = (Q K^T)^T computed as lhsT=K^T-chunk [d,128], rhs=Q^T [d,512].
  exp on ScalarE (scale folded in) writes P^T [k, q] straight into the
  layout the PV matmul wants as its moving operand; V needs no transpose.
  out^T[d, q] accumulates over k-chunks in PSUM; denominator accumulates
  via a ones[128,1] matmul. Causal masking = column-restricted matmuls +
  one [128,128] mask add on diagonal blocks.

The host pre-transposes the shards so the device needs zero PE
transposes: q arrives as [B, HL, D, QL], k as [B, D, KL], and the
output leaves the device as [B, HL, D, QL] (host transposes it back).
"""

import math

import numpy as np

import concourse.bass as bass
import concourse.tile as tile
from concourse import bacc, mybir
from concourse.bass_utils import run_bass_kernel_spmd

P = 128
F32 = mybir.dt.float32

# Matmul operand dtype: float32 is exact (4 cyc/row), float32r is a
# TF32-like rounded mode (1 cyc/row at N>=256). Measured end-to-end
# error vs the fp32 reference: fp32 4.6e-6 rel, f32r 2.9e-4 rel.
MM_DT = mybir.dt.float32r
# Normalization path: "gpsimd_bcast" uses the GpSimd partition-broadcast
# custom instruction; "pe_bcast" broadcasts recip(den) across partitions
# via a PE outer product instead.
NORM_MODE = "gpsimd_bcast"
MASK_VAL = -1e6
# timing-only ablations (wrong numerics when set)
ABLATE = set()
# scheduling knobs
PIPE_DEPTH = 4
ST1 = 3       # k-chunk index at which the deferred store flushes
QLD = 2       # k-chunk index at which next tile's Q^T load is emitted
BUFS = {"ps_s": 5, "ps_o": 2, "ps_d": 1, "sb_pt": 5}


def emit_attention(nc, tc, ctx, q_ap, k_ap, v_ap, o_ap, B, QL, KL, HL, D,
                   dt_mm=None):
    """Emit the per-core attention program.

    q_ap/o_ap: [B, HL, D, QL] dram APs; k_ap: [B, D, KL]; v_ap: [B, KL, D].
    """
    assert D == P
    dt_mm = dt_mm or MM_DT
    QT = 512                       # q tile (psum bank = 512 fp32)
    KC = P                         # k chunk (contraction granularity)
    n_qt = QL // QT
    n_kc_total = KL // KC
    qt_per_kc = QT // KC           # k-chunks per q-tile (diag straddle count)
    scale = 1.0 / math.sqrt(D)

    sb = ctx.enter_context(tc.tile_pool(name="sb", bufs=1))
    sb_q = ctx.enter_context(tc.tile_pool(name="sb_q", bufs=2))
    sb_pt = ctx.enter_context(tc.tile_pool(name="sb_pt", bufs=BUFS["sb_pt"]))
    sb_o = ctx.enter_context(tc.tile_pool(name="sb_o", bufs=2))
    ps_s = ctx.enter_context(tc.tile_pool(name="ps_s", bufs=BUFS["ps_s"], space="PSUM"))
    ps_o = ctx.enter_context(tc.tile_pool(name="ps_o", bufs=BUFS["ps_o"], space="PSUM"))
    ps_d = ctx.enter_context(tc.tile_pool(name="ps_d", bufs=BUFS["ps_d"], space="PSUM"))

    # --- constants ---
    # causal mask in S^T coords: [k_local, q_local], fill where q < k
    mask = sb.tile([P, P], F32, name="mask")
    nc.gpsimd.memset(mask[:], 0.0)
    nc.gpsimd.affine_select(
        out=mask[:], in_=mask[:], compare_op=mybir.AluOpType.is_ge,
        fill=MASK_VAL, base=0, pattern=[[1, P]], channel_multiplier=-1,
    )
    ones_f32 = sb.tile([P, 1], F32, name="ones_f32")
    nc.gpsimd.memset(ones_f32[:], 1.0)
    ones = sb.tile([P, 1], dt_mm, name="ones")
    nc.vector.tensor_copy(ones[:], ones_f32[:])
    onesrow = sb.tile([1, P], F32, name="onesrow")
    nc.gpsimd.memset(onesrow[:], 1.0)

    # --- per-batch K^T and V, resident in SBUF ---
    KTs, Vs = [], []
    for b in range(B):
        kstage = sb.tile([P, KL], F32, name=f"kstage{b}")
        nc.sync.dma_start(kstage[:], k_ap[b])
        KT = sb.tile([P, KL], dt_mm, name=f"KT{b}")
        nc.vector.tensor_copy(KT[:], kstage[:])
        KTs.append(KT)

        vstage = sb.tile([P, n_kc_total, P], F32, name=f"vstage{b}")
        nc.sync.dma_start(
            vstage[:], v_ap[b].rearrange("(c p) d -> p c d", p=P))
        V = sb.tile([P, n_kc_total, P], dt_mm, name=f"V{b}")
        nc.vector.tensor_copy(V[:], vstage[:])
        Vs.append(V)

    # --- main loops ---
    # Epilogues are deferred by one q-tile: the den/normalize/transpose
    # chain of tile t runs while tile t+1's matmuls stream, so PE never
    # waits on it. `pending` holds (stage, closure) pairs. The Q^T load
    # for tile t+1 is likewise emitted mid-way through tile t's k-loop.
    pending = []

    def flush_pending(stage):
        while pending and pending[0][0] <= stage:
            pending.pop(0)[1]()

    def emit_qload(b, h, qt):
        q0 = qt * QT
        qstage = sb_q.tile([P, QT], F32, tag="qstage")
        nc.sync.dma_start(qstage[:], q_ap[b, h, :, q0:q0 + QT])
        QTt = sb_q.tile([P, QT], dt_mm, tag="qtt")
        nc.vector.tensor_copy(QTt[:], qstage[:])
        return QTt

    items = [(b, h, qt) for b in range(B) for h in range(HL)
             for qt in range(n_qt)]
    qtt_next = emit_qload(*items[0])

    for it, (b, h, qt) in enumerate(items):
        for _only_once in (0,):
                q0 = qt * QT
                n_kc = (q0 + QT) // KC
                QTt = qtt_next

                O_ps = ps_o.tile([P, QT], F32, tag="o")
                D_ps = ps_d.tile([1, QT], F32, tag="d")

                # software-pipelined k-chunk loop: S(kc) runs PIPE ahead
                # of O/D(kc) so PE never stalls on the ACT exp.
                PIPE = PIPE_DEPTH
                pts = {}

                def emit_S(kc, qt=qt, b=b, QTt=QTt, pts=pts):
                    diag_j = kc - qt * qt_per_kc
                    col0 = max(0, diag_j) * KC if diag_j >= 0 else 0
                    S_ps = ps_s.tile([P, QT], F32, tag="s")
                    nc.tensor.matmul(
                        S_ps[:, col0:], KTs[b][:, kc * KC:(kc + 1) * KC],
                        QTt[:, col0:], start=True, stop=True,
                        skip_group_check=True)
                    if diag_j >= 0 and "mask" not in ABLATE:
                        nc.vector.tensor_add(
                            S_ps[:, col0:col0 + P], S_ps[:, col0:col0 + P],
                            mask[:])
                    PT = sb_pt.tile([P, QT], dt_mm, tag="pt")
                    if "exp" in ABLATE:
                        nc.vector.tensor_copy(PT[:, col0:], S_ps[:, col0:])
                    else:
                        nc.scalar.activation(
                            PT[:, col0:], S_ps[:, col0:],
                            mybir.ActivationFunctionType.Exp, scale=scale)
                    pts[kc] = (PT, col0)

                def emit_OD(kc, n_kc=n_kc, O_ps=O_ps, D_ps=D_ps, b=b,
                            pts=pts):
                    PT, col0 = pts.pop(kc)
                    first, last = kc == 0, kc == n_kc - 1
                    nc.tensor.matmul(
                        O_ps[:, col0:], Vs[b][:, kc, :], PT[:, col0:],
                        start=first, stop=last, skip_group_check=True)
                    if "den" not in ABLATE:
                        nc.tensor.matmul(
                            D_ps[:1, col0:], ones[:, :1], PT[:, col0:],
                            start=first, stop=last, skip_group_check=True)

                st1 = min(ST1, n_kc - 1)
                qld = min(QLD, n_kc - 2)
                for kc in range(n_kc):
                    emit_S(kc)
                    if kc == 1:
                        flush_pending(0)
                    if kc == st1:
                        flush_pending(1)
                    if kc == qld and it + 1 < len(items):
                        qtt_next = emit_qload(*items[it + 1])
                    if kc >= PIPE:
                        emit_OD(kc - PIPE)
                for kc in range(max(0, n_kc - PIPE), n_kc):
                    emit_OD(kc)

                def emit_norm(O_ps=O_ps, D_ps=D_ps):
                    den = sb_o.tile([1, QT], F32, tag="den")
                    if "den" not in ABLATE:
                        nc.vector.tensor_copy(den[:], D_ps[:1, :])
                        nc.vector.reciprocal(den[:], den[:])
                    O_sb = sb_o.tile([P, QT], F32, tag="osb")
                    if "norm" in ABLATE:
                        nc.vector.tensor_copy(O_sb[:], O_ps[:])
                    else:
                        bc_sb = sb_o.tile([P, QT], F32, tag="bcsb")
                        nc.gpsimd.partition_broadcast(bc_sb[:], den[:])
                        nc.vector.tensor_mul(O_sb[:], O_ps[:], bc_sb[:])
                    return O_sb

                def emit_store(O_sb, b=b, h=h, q0=q0):
                    nc.sync.dma_start(o_ap[b, h, :, q0:q0 + QT], O_sb[:])

                def epi(emit_norm=emit_norm, emit_store=emit_store):
                    state = {}

                    def norm_part():
                        state["O_sb"] = emit_norm()

                    def store_part():
                        emit_store(state["O_sb"])
                    return norm_part, store_part

                norm_part, store_part = epi()
                pending.append((0, norm_part))
                pending.append((1, store_part))
    flush_pending(1)


def build_nc(B=2, QL=2048, KL=2048, HL=4, D=128, dt_mm=None, reps=1):
    nc = bacc.Bacc("TRN2", target_bir_lowering=False, debug=False,
                   num_devices=8)
    q = nc.dram_tensor("q", [B, HL, D, QL], F32, kind="ExternalInput")
    k = nc.dram_tensor("k", [B, D, KL], F32, kind="ExternalInput")
    v = nc.dram_tensor("v", [B, KL, D], F32, kind="ExternalInput")
    o = nc.dram_tensor("out", [B, HL, D, QL], F32, kind="ExternalOutput")
    from contextlib import ExitStack
    with tile.TileContext(nc) as tc:
        for _ in range(reps):
            with ExitStack() as ctx:
                emit_attention(nc, tc, ctx, q.ap(), k.ap(), v.ap(), o.ap(),
                               B, QL, KL, HL, D, dt_mm=dt_mm)
    nc.compile()
    return nc


_NC_CACHE = {}


def kernel(q: np.ndarray, k: np.ndarray, v: np.ndarray) -> np.ndarray:
    B, QL, H, D = q.shape
    KL, KVH = k.shape[1], k.shape[2]
    n_cores = 8
    HL = H // n_cores            # q-heads per core (4)
    assert KVH == n_cores and H == 32 and D == 128

    if "nc" not in _NC_CACHE:
        _NC_CACHE["nc"] = build_nc(B=B, QL=QL, KL=KL, HL=HL, D=D)
    nc = _NC_CACHE["nc"]

    q = np.asarray(q, dtype=np.float32)
    k = np.asarray(k, dtype=np.float32)
    v = np.asarray(v, dtype=np.float32)
    in_maps = []
    for c in range(n_cores):
        in_maps.append({
            # [B, HL, D, QL]
            "q": np.ascontiguousarray(
                q[:, :, HL * c:HL * (c + 1), :].transpose(0, 2, 3, 1)),
            # [B, D, KL]
            "k": np.ascontiguousarray(k[:, :, c, :].transpose(0, 2, 1)),
            "v": np.ascontiguousarray(v[:, :, c, :]),
        })
    res = run_bass_kernel_spmd(nc, in_maps, list(range(n_cores)))
    # device output is [B, HL, D, QL] per core -> [B, QL, HL, D], concat heads
    return np.concatenate(
        [r["out"].transpose(0, 3, 1, 2) for r in res.results], axis=2)


# revision 25
# speedup vs baseline: 21514.8960x; 19570.8852x over previous
"""Causal GQA attention for Trainium2, sharded across 8 NeuronCores.

Problem: q [2, 2048, 32, 128], k/v [2, 2048, 8, 128] fp32, causal,
GQA group = 4. Sharding: core i gets kv-head i plus its 4 q-heads
(heads 4i..4i+3), both batch elements. Each core runs the same program
(SPMD) on its shard; outputs are concatenated on the head axis.

Per-core kernel (flash-attention style, transposed-S layout):
  S^T[k, q] = (Q K^T)^T computed as lhsT=K^T-chunk [d,128], rhs=Q^T [d,512].
  exp on ScalarE (scale folded in) writes P^T [k, q] straight into the
  layout the PV matmul wants as its moving operand; V needs no transpose.
  out^T[d, q] accumulates over k-chunks in PSUM; denominator accumulates
  via a ones[128,1] matmul. Causal masking = column-restricted matmuls +
  one [128,128] mask add on diagonal blocks.

The host pre-arranges the shards so the device needs zero PE transposes
and few DMA descriptors: q arrives as [B, HL, D, QL], k as [B, D, KL],
v as [B, 128, KL/128, 128] (chunk-swizzled), and the output leaves the
device as [B, HL, D, QL] (the host transposes it back). All DMAs move
>=2 KiB contiguous runs per partition, with one q-load and one
out-store per (b, h).
"""

import math

import numpy as np

import concourse.tile as tile
from concourse import bacc, mybir
from concourse.bass_utils import run_bass_kernel_spmd

P = 128
F32 = mybir.dt.float32

# Matmul operand dtype: float32 is exact (4 cyc/row), float32r is a
# TF32-like rounded mode (1 cyc/row at N>=256). Measured end-to-end
# error vs the fp32 reference: fp32 4.6e-6 rel, f32r 2.9e-4 rel.
MM_DT = mybir.dt.float32r
MASK_VAL = -1e6
# timing-only ablations (wrong numerics when set)
ABLATE = set()
# scheduling knobs
PIPE_DEPTH = 4
EXP_PAIR = False  # one ACT exp per two k-chunks (double-width S psum)
BUFS = {"ps_s": 5, "ps_o": 2, "ps_d": 1, "sb_pt": 5,
        "ps_s2": 2, "sb_pt2": 3}


def emit_attention(nc, tc, ctx, q_ap, k_ap, v_ap, o_ap, B, QL, KL, HL, D,
                   dt_mm=None):
    """Emit the per-core attention program.

    q_ap/o_ap: [B, HL, D, QL]; k_ap: [B, D, KL]; v_ap: [B, P, KL/P, P].
    """
    assert D == P
    dt_mm = dt_mm or MM_DT
    QT = 512                       # q tile (psum bank = 512 fp32)
    KC = P                         # k chunk (contraction granularity)
    n_qt = QL // QT
    n_kc_total = KL // KC
    qt_per_kc = QT // KC
    scale = 1.0 / math.sqrt(D)

    sb = ctx.enter_context(tc.tile_pool(name="sb", bufs=1))
    sb_q = ctx.enter_context(tc.tile_pool(name="sb_q", bufs=2))
    sb_pt = ctx.enter_context(tc.tile_pool(name="sb_pt", bufs=BUFS["sb_pt"]))
    sb_o = ctx.enter_context(tc.tile_pool(name="sb_o", bufs=2))
    ps_s = ctx.enter_context(
        tc.tile_pool(name="ps_s",
                     bufs=BUFS["ps_s2"] if EXP_PAIR else BUFS["ps_s"],
                     space="PSUM"))
    ps_o = ctx.enter_context(
        tc.tile_pool(name="ps_o", bufs=BUFS["ps_o"], space="PSUM"))
    ps_d = ctx.enter_context(
        tc.tile_pool(name="ps_d", bufs=BUFS["ps_d"], space="PSUM"))

    # --- constants ---
    # causal mask in S^T coords: [k_local, q_local], fill where q < k
    mask = sb.tile([P, P], F32, name="mask")
    nc.gpsimd.memset(mask[:], 0.0)
    nc.gpsimd.affine_select(
        out=mask[:], in_=mask[:], compare_op=mybir.AluOpType.is_ge,
        fill=MASK_VAL, base=0, pattern=[[1, P]], channel_multiplier=-1,
    )
    ones_f32 = sb.tile([P, 1], F32, name="ones_f32")
    nc.gpsimd.memset(ones_f32[:], 1.0)
    ones = sb.tile([P, 1], dt_mm, name="ones")
    nc.vector.tensor_copy(ones[:], ones_f32[:])
    negblk = sb.tile([P, P], F32, name="negblk")
    nc.gpsimd.memset(negblk[:], MASK_VAL)

    # first Q^T load goes out before the K/V preamble so its DMA isn't
    # queued behind ~4 MB of K/V traffic on the same engine
    def emit_qload(b, h):
        qstage = sb_q.tile([P, QL], F32, tag="qstage")
        nc.sync.dma_start(qstage[:], q_ap[b, h])
        QTt = sb_q.tile([P, QL], dt_mm, tag="qtt")
        nc.vector.tensor_copy(QTt[:], qstage[:])
        return QTt

    qtt_cur = emit_qload(0, 0)

    # --- per-batch K^T and V, resident in SBUF (chunked loads so the
    # first S-matmul starts after ~1/4 of the DMA) ---
    KTs, Vs = [], []
    G = 4
    for b in range(B):
        kstage = sb.tile([P, KL], F32, name=f"kstage{b}")
        KT = sb.tile([P, KL], dt_mm, name=f"KT{b}")
        vstage = sb.tile([P, n_kc_total, P], F32, name=f"vstage{b}")
        V = sb.tile([P, n_kc_total, P], dt_mm, name=f"V{b}")
        gk = KL // G
        gc = n_kc_total // G
        for g in range(G):
            nc.sync.dma_start(kstage[:, g * gk:(g + 1) * gk],
                              k_ap[b][:, g * gk:(g + 1) * gk])
            nc.vector.tensor_copy(KT[:, g * gk:(g + 1) * gk],
                                  kstage[:, g * gk:(g + 1) * gk])
            nc.sync.dma_start(vstage[:, g * gc:(g + 1) * gc, :],
                              v_ap[b][:, g * gc:(g + 1) * gc, :])
            nc.vector.tensor_copy(V[:, g * gc:(g + 1) * gc, :],
                                  vstage[:, g * gc:(g + 1) * gc, :])
        KTs.append(KT)
        Vs.append(V)

    # --- main loops ---
    # One q-load and one out-store per (b, h) (128 descriptors each).
    # Epilogues are deferred by one q-tile so PE never waits on them.
    pending = []

    def flush_pending(stage):
        while pending and pending[0][0] <= stage:
            pending.pop(0)[1]()

    heads = [(b, h) for b in range(B) for h in range(HL)]
    items = [(bi, qt) for bi in range(len(heads)) for qt in range(n_qt)]
    qtt_next = None
    obh_cur = sb_o.tile([P, QL], F32, tag="obh")
    obh_next = None

    for it, (bi, qt) in enumerate(items):
        b, h = heads[bi]
        q0 = qt * QT
        n_kc = (q0 + QT) // KC
        QTt, Obh = qtt_cur, obh_cur

        O_ps = ps_o.tile([P, QT], F32, tag="o")
        D_ps = ps_d.tile([1, QT], F32, tag="d")

        # software-pipelined k-chunk loop: S(kc) runs PIPE ahead of
        # O/D(kc) so PE never stalls on the ACT exp.
        PIPE = PIPE_DEPTH
        pts = {}

        def s_cols(kc, qt=qt):
            diag_j = kc - qt * qt_per_kc
            return max(0, diag_j) * KC if diag_j >= 0 else 0, diag_j

        def emit_S_into(S_view, kc, col0, diag_j, b=b, QTt=QTt, q0=q0):
            nc.tensor.matmul(
                S_view[:, col0:], KTs[b][:, kc * KC:(kc + 1) * KC],
                QTt[:, q0 + col0:q0 + QT], start=True, stop=True,
                skip_group_check=True)
            if diag_j >= 0 and "mask" not in ABLATE:
                nc.vector.tensor_add(
                    S_view[:, col0:col0 + P], S_view[:, col0:col0 + P],
                    mask[:])

        def emit_S(kc, pts=pts):
            col0, diag_j = s_cols(kc)
            S_ps = ps_s.tile([P, QT], F32, tag="s")
            emit_S_into(S_ps, kc, col0, diag_j)
            PT = sb_pt.tile([P, QT], dt_mm, tag="pt")
            nc.scalar.activation(
                PT[:, col0:], S_ps[:, col0:],
                mybir.ActivationFunctionType.Exp, scale=scale)
            pts[kc] = (PT, col0)

        def emit_S_pair(pc, pts=pts):
            kc0, kc1 = 2 * pc, 2 * pc + 1
            c0, dj0 = s_cols(kc0)
            c1, dj1 = s_cols(kc1)
            S2 = ps_s.tile([P, 2, QT], F32, tag="s")
            emit_S_into(S2[:, 0, :], kc0, c0, dj0)
            emit_S_into(S2[:, 1, :], kc1, c1, dj1)
            if c1 > c0:
                # blank the late-start gap of the second half so the
                # pair-wide exp writes zeros there
                nc.vector.tensor_copy(S2[:, 1, c0:c1], negblk[:, :c1 - c0])
            PT2 = sb_pt.tile([P, 2, QT], dt_mm, tag="pt")
            nc.scalar.activation(
                PT2[:, :, c0:], S2[:, :, c0:],
                mybir.ActivationFunctionType.Exp, scale=scale)
            pts[kc0] = (PT2[:, 0, :], c0)
            pts[kc1] = (PT2[:, 1, :], c1)

        def emit_OD(kc, n_kc=n_kc, O_ps=O_ps, D_ps=D_ps, b=b, pts=pts):
            PT, col0 = pts.pop(kc)
            first, last = kc == 0, kc == n_kc - 1
            nc.tensor.matmul(
                O_ps[:, col0:], Vs[b][:, kc, :], PT[:, col0:],
                start=first, stop=last, skip_group_check=True)
            if "den" not in ABLATE:
                nc.tensor.matmul(
                    D_ps[:1, col0:], ones[:, :1], PT[:, col0:],
                    start=first, stop=last, skip_group_check=True)

        if EXP_PAIR:
            n_pc = n_kc // 2
            for pc in range(n_pc):
                emit_S_pair(pc)
                if pc == 0:
                    flush_pending(0)
                if pc == min(1, n_pc - 1):
                    flush_pending(1)
                if pc == min(1, n_pc - 1) and qt == 0 and bi + 1 < len(heads):
                    qtt_next = emit_qload(*heads[bi + 1])
                    obh_next = sb_o.tile([P, QL], F32, tag="obh")
                if pc >= 2:
                    emit_OD(2 * (pc - 2))
                    emit_OD(2 * (pc - 2) + 1)
            for kc in range(max(0, n_kc - 4), n_kc):
                emit_OD(kc)
        else:
            st1 = min(3, n_kc - 1)
            qld = min(2, n_kc - 2)
            for kc in range(n_kc):
                emit_S(kc)
                if kc == 1:
                    flush_pending(0)
                if kc == st1:
                    flush_pending(1)
                if kc == qld and qt == 0 and bi + 1 < len(heads):
                    qtt_next = emit_qload(*heads[bi + 1])
                    obh_next = sb_o.tile([P, QL], F32, tag="obh")
                if kc >= PIPE:
                    emit_OD(kc - PIPE)
            for kc in range(max(0, n_kc - PIPE), n_kc):
                emit_OD(kc)

        def emit_norm(O_ps=O_ps, D_ps=D_ps, Obh=Obh, q0=q0):
            den = sb_o.tile([1, QT], F32, tag="den")
            if "den" not in ABLATE:
                nc.vector.tensor_copy(den[:], D_ps[:1, :])
                nc.vector.reciprocal(den[:], den[:])
            if "norm" in ABLATE:
                nc.vector.tensor_copy(Obh[:, q0:q0 + QT], O_ps[:])
            else:
                bc_sb = sb_o.tile([P, QT], F32, tag="bcsb")
                nc.gpsimd.partition_broadcast(bc_sb[:], den[:])
                nc.vector.tensor_mul(Obh[:, q0:q0 + QT], O_ps[:], bc_sb[:])

        pending.append((0, emit_norm))
        if qt == n_qt - 1:
            def emit_store(Obh=Obh, b=b, h=h):
                nc.scalar.dma_start(o_ap[b, h], Obh[:])
            pending.append((1, emit_store))
            qtt_cur, obh_cur = qtt_next, obh_next
    flush_pending(1)


def build_nc(B=2, QL=2048, KL=2048, HL=4, D=128, dt_mm=None, reps=1):
    nc = bacc.Bacc("TRN2", target_bir_lowering=False, debug=False,
                   num_devices=8)
    q = nc.dram_tensor("q", [B, HL, D, QL], F32, kind="ExternalInput")
    k = nc.dram_tensor("k", [B, D, KL], F32, kind="ExternalInput")
    v = nc.dram_tensor("v", [B, P, KL // P, P], F32, kind="ExternalInput")
    o = nc.dram_tensor("out", [B, HL, D, QL], F32, kind="ExternalOutput")
    from contextlib import ExitStack
    with tile.TileContext(nc) as tc:
        for _ in range(reps):
            with ExitStack() as ctx:
                emit_attention(nc, tc, ctx, q.ap(), k.ap(), v.ap(), o.ap(),
                               B, QL, KL, HL, D, dt_mm=dt_mm)
    nc.compile()
    return nc


def shard_inputs(q, k, v, n_cores=8):
    B, QL, H, D = q.shape
    KL = k.shape[1]
    HL = H // n_cores
    in_maps = []
    for c in range(n_cores):
        in_maps.append({
            # [B, HL, D, QL]
            "q": np.ascontiguousarray(
                q[:, :, HL * c:HL * (c + 1), :].transpose(0, 2, 3, 1)),
            # [B, D, KL]
            "k": np.ascontiguousarray(k[:, :, c, :].transpose(0, 2, 1)),
            # [B, P, KL/P, P]: partition = position within a 128-chunk
            "v": np.ascontiguousarray(
                v[:, :, c, :].reshape(B, KL // P, P, D).transpose(0, 2, 1, 3)),
        })
    return in_maps


_NC_CACHE = {}


def kernel(q: np.ndarray, k: np.ndarray, v: np.ndarray) -> np.ndarray:
    B, QL, H, D = q.shape
    KL, KVH = k.shape[1], k.shape[2]
    n_cores = 8
    HL = H // n_cores            # q-heads per core (4)
    assert KVH == n_cores and H == 32 and D == 128

    if "nc" not in _NC_CACHE:
        _NC_CACHE["nc"] = build_nc(B=B, QL=QL, KL=KL, HL=HL, D=D)
    nc = _NC_CACHE["nc"]

    q = np.asarray(q, dtype=np.float32)
    k = np.asarray(k, dtype=np.float32)
    v = np.asarray(v, dtype=np.float32)
    in_maps = shard_inputs(q, k, v, n_cores)
    res = run_bass_kernel_spmd(nc, in_maps, list(range(n_cores)))
    # device output is [B, HL, D, QL] per core -> [B, QL, HL, D], concat heads
    return np.concatenate(
        [r["out"].transpose(0, 3, 1, 2) for r in res.results], axis=2)


# revision 29
# speedup vs baseline: 33369.9814x; 1.5510x over previous
"""Causal GQA attention for Trainium2, sharded across 8 NeuronCores.

Problem: q [2, 2048, 32, 128], k/v [2, 2048, 8, 128] fp32, causal,
GQA group = 4. Sharding: core i gets kv-head i plus its 4 q-heads
(heads 4i..4i+3), both batch elements. Each core runs the same program
(SPMD) on its shard; outputs are concatenated on the head axis.

Per-core kernel (flash-attention style, transposed-S layout):
  S^T[k, q] = (Q K^T)^T computed as lhsT=K^T-chunk [d,128], rhs=Q^T [d,512].
  exp on ScalarE (scale folded in) writes P^T [k, q] straight into the
  layout the PV matmul wants as its moving operand; V needs no transpose.
  out^T[d, q] accumulates over k-chunks in PSUM; denominator accumulates
  via a ones[128,1] matmul. Causal masking = column-restricted matmuls +
  one [128,128] mask add on diagonal blocks.

The host pre-arranges the shards so the device needs zero PE transposes
and few DMA descriptors: q arrives as [B, HL, D, QL], k as [B, D, KL],
v as [B, 128, KL/128, 128] (chunk-swizzled), and the output leaves the
device as [B, HL, D, QL] (the host transposes it back). All DMAs move
>=2 KiB contiguous runs per partition, with one q-load and one
out-store per (b, h).
"""

import math

import numpy as np

import concourse.tile as tile
from concourse import bacc, mybir
from concourse.bass_utils import run_bass_kernel_spmd

P = 128
F32 = mybir.dt.float32

# Matmul operand dtype: float32 is exact (4 cyc/row), float32r is a
# TF32-like rounded mode (1 cyc/row at N>=256). Measured end-to-end
# error vs the fp32 reference: fp32 4.6e-6 rel, f32r 2.9e-4 rel.
MM_DT = mybir.dt.float32r
MASK_VAL = -1e6
# timing-only ablations (wrong numerics when set)
ABLATE = set()
# scheduling knobs
PIPE_DEPTH = 4
MASK_POST = True  # mask by zeroing P^T post-exp in SBUF (no PSUM RMW)
DEN_BCAST = True  # den matmul uses ones[128,128]: result lands pre-broadcast
EXP_PAIR = False  # one ACT exp per two k-chunks (double-width S psum)
BUFS = {"ps_s": 5, "ps_o": 2, "ps_d": 1, "sb_pt": 5,
        "ps_s2": 2, "sb_pt2": 3}


def emit_attention(nc, tc, ctx, q_ap, k_ap, v_ap, o_ap, B, QL, KL, HL, D,
                   dt_mm=None):
    """Emit the per-core attention program.

    q_ap/o_ap: [B, HL, D, QL]; k_ap: [B, D, KL]; v_ap: [B, P, KL/P, P].
    """
    assert D == P
    dt_mm = dt_mm or MM_DT
    QT = 512                       # q tile (psum bank = 512 fp32)
    KC = P                         # k chunk (contraction granularity)
    n_qt = QL // QT
    n_kc_total = KL // KC
    qt_per_kc = QT // KC
    scale = 1.0 / math.sqrt(D)

    sb = ctx.enter_context(tc.tile_pool(name="sb", bufs=1))
    sb_q = ctx.enter_context(tc.tile_pool(name="sb_q", bufs=2))
    sb_pt = ctx.enter_context(tc.tile_pool(name="sb_pt", bufs=BUFS["sb_pt"]))
    sb_o = ctx.enter_context(tc.tile_pool(name="sb_o", bufs=2))
    ps_s = ctx.enter_context(
        tc.tile_pool(name="ps_s",
                     bufs=BUFS["ps_s2"] if EXP_PAIR else BUFS["ps_s"],
                     space="PSUM"))
    ps_o = ctx.enter_context(
        tc.tile_pool(name="ps_o", bufs=BUFS["ps_o"], space="PSUM"))
    ps_d = ctx.enter_context(
        tc.tile_pool(name="ps_d", bufs=BUFS["ps_d"], space="PSUM"))

    # --- constants ---
    # causal mask in S^T coords: [k_local, q_local], fill where q < k
    mask = sb.tile([P, P], F32, name="mask")
    nc.gpsimd.memset(mask[:], 0.0)
    nc.gpsimd.affine_select(
        out=mask[:], in_=mask[:], compare_op=mybir.AluOpType.is_ge,
        fill=MASK_VAL, base=0, pattern=[[1, P]], channel_multiplier=-1,
    )
    ones_f32 = sb.tile([P, P], F32, name="ones_f32")
    nc.gpsimd.memset(ones_f32[:], 1.0)
    ones = sb.tile([P, P], dt_mm, name="ones")
    nc.vector.tensor_copy(ones[:], ones_f32[:])
    negblk = sb.tile([P, P], F32, name="negblk")
    nc.gpsimd.memset(negblk[:], MASK_VAL)
    # 0/1 causal mask for the post-exp variant: 1 where q >= k
    mask01 = sb.tile([P, P], F32, name="mask01")
    nc.gpsimd.memset(mask01[:], 1.0)
    nc.gpsimd.affine_select(
        out=mask01[:], in_=mask01[:], compare_op=mybir.AluOpType.is_ge,
        fill=0.0, base=0, pattern=[[1, P]], channel_multiplier=-1,
    )

    # first Q^T load goes out before the K/V preamble so its DMA isn't
    # queued behind ~4 MB of K/V traffic on the same engine
    def emit_qload(b, h, split=False):
        qstage = sb_q.tile([P, QL], F32, tag="qstage")
        QTt = sb_q.tile([P, QL], dt_mm, tag="qtt")
        if split:
            # first 512 cols land first so S(0) can start ~4us earlier
            nc.sync.dma_start(qstage[:, :QT], q_ap[b, h][:, :QT])
            nc.vector.tensor_copy(QTt[:, :QT], qstage[:, :QT])
            nc.sync.dma_start(qstage[:, QT:], q_ap[b, h][:, QT:])
            nc.vector.tensor_copy(QTt[:, QT:], qstage[:, QT:])
        else:
            nc.sync.dma_start(qstage[:], q_ap[b, h])
            nc.vector.tensor_copy(QTt[:], qstage[:])
        return QTt

    qtt_cur = emit_qload(0, 0)

    # --- per-batch K^T and V, resident in SBUF (chunked loads so the
    # first S-matmul starts after ~1/4 of the DMA) ---
    KTs, Vs = [], []
    G = 4
    for b in range(B):
        kstage = sb.tile([P, KL], F32, name=f"kstage{b}")
        KT = sb.tile([P, KL], dt_mm, name=f"KT{b}")
        vstage = sb.tile([P, n_kc_total, P], F32, name=f"vstage{b}")
        V = sb.tile([P, n_kc_total, P], dt_mm, name=f"V{b}")
        gk = KL // G
        gc = n_kc_total // G
        for g in range(G):
            nc.sync.dma_start(kstage[:, g * gk:(g + 1) * gk],
                              k_ap[b][:, g * gk:(g + 1) * gk])
            nc.vector.tensor_copy(KT[:, g * gk:(g + 1) * gk],
                                  kstage[:, g * gk:(g + 1) * gk])
            nc.sync.dma_start(vstage[:, g * gc:(g + 1) * gc, :],
                              v_ap[b][:, g * gc:(g + 1) * gc, :])
            nc.vector.tensor_copy(V[:, g * gc:(g + 1) * gc, :],
                                  vstage[:, g * gc:(g + 1) * gc, :])
        KTs.append(KT)
        Vs.append(V)

    # --- main loops ---
    # One q-load and one out-store per (b, h) (128 descriptors each).
    # Epilogues are deferred by one q-tile so PE never waits on them.
    pending = []

    def flush_pending(stage):
        while pending and pending[0][0] <= stage:
            pending.pop(0)[1]()

    heads = [(b, h) for b in range(B) for h in range(HL)]
    items = [(bi, qt) for bi in range(len(heads)) for qt in range(n_qt)]
    qtt_next = None
    obh_cur = sb_o.tile([P, QL], F32, tag="obh")
    obh_next = None

    for it, (bi, qt) in enumerate(items):
        b, h = heads[bi]
        q0 = qt * QT
        n_kc = (q0 + QT) // KC
        QTt, Obh = qtt_cur, obh_cur

        O_ps = ps_o.tile([P, QT], F32, tag="o")
        D_ps = ps_d.tile([P if DEN_BCAST else 1, QT], F32, tag="d")

        # software-pipelined k-chunk loop: S(kc) runs PIPE ahead of
        # O/D(kc) so PE never stalls on the ACT exp.
        PIPE = PIPE_DEPTH
        pts = {}

        def s_cols(kc, qt=qt):
            diag_j = kc - qt * qt_per_kc
            return max(0, diag_j) * KC if diag_j >= 0 else 0, diag_j

        def emit_S_into(S_view, kc, col0, diag_j, b=b, QTt=QTt, q0=q0):
            nc.tensor.matmul(
                S_view[:, col0:], KTs[b][:, kc * KC:(kc + 1) * KC],
                QTt[:, q0 + col0:q0 + QT], start=True, stop=True,
                skip_group_check=True)
            if diag_j >= 0 and "mask" not in ABLATE and not MASK_POST:
                nc.vector.tensor_add(
                    S_view[:, col0:col0 + P], S_view[:, col0:col0 + P],
                    mask[:])

        def emit_S(kc, pts=pts):
            col0, diag_j = s_cols(kc)
            S_ps = ps_s.tile([P, QT], F32, tag="s")
            emit_S_into(S_ps, kc, col0, diag_j)
            PT = sb_pt.tile([P, QT], dt_mm, tag="pt")
            nc.scalar.activation(
                PT[:, col0:], S_ps[:, col0:],
                mybir.ActivationFunctionType.Exp, scale=scale)
            if diag_j >= 0 and "mask" not in ABLATE and MASK_POST:
                nc.vector.tensor_mul(
                    PT[:, col0:col0 + P], PT[:, col0:col0 + P], mask01[:])
            pts[kc] = (PT, col0)

        def emit_S_pair(pc, pts=pts):
            kc0, kc1 = 2 * pc, 2 * pc + 1
            c0, dj0 = s_cols(kc0)
            c1, dj1 = s_cols(kc1)
            S2 = ps_s.tile([P, 2, QT], F32, tag="s")
            emit_S_into(S2[:, 0, :], kc0, c0, dj0)
            emit_S_into(S2[:, 1, :], kc1, c1, dj1)
            if c1 > c0:
                # blank the late-start gap of the second half so the
                # pair-wide exp writes zeros there
                nc.vector.tensor_copy(S2[:, 1, c0:c1], negblk[:, :c1 - c0])
            PT2 = sb_pt.tile([P, 2, QT], dt_mm, tag="pt")
            nc.scalar.activation(
                PT2[:, :, c0:], S2[:, :, c0:],
                mybir.ActivationFunctionType.Exp, scale=scale)
            pts[kc0] = (PT2[:, 0, :], c0)
            pts[kc1] = (PT2[:, 1, :], c1)

        def emit_OD(kc, n_kc=n_kc, O_ps=O_ps, D_ps=D_ps, b=b, pts=pts):
            PT, col0 = pts.pop(kc)
            first, last = kc == 0, kc == n_kc - 1
            nc.tensor.matmul(
                O_ps[:, col0:], Vs[b][:, kc, :], PT[:, col0:],
                start=first, stop=last, skip_group_check=True)
            if "den" not in ABLATE:
                nc.tensor.matmul(
                    D_ps[:, col0:] if DEN_BCAST else D_ps[:1, col0:],
                    ones[:, :] if DEN_BCAST else ones[:, :1],
                    PT[:, col0:],
                    start=first, stop=last, skip_group_check=True)

        if EXP_PAIR:
            n_pc = n_kc // 2
            for pc in range(n_pc):
                emit_S_pair(pc)
                if pc == 0:
                    flush_pending(0)
                if pc == min(1, n_pc - 1):
                    flush_pending(1)
                if pc == min(1, n_pc - 1) and qt == 0 and bi + 1 < len(heads):
                    qtt_next = emit_qload(*heads[bi + 1])
                    obh_next = sb_o.tile([P, QL], F32, tag="obh")
                if pc >= 2:
                    emit_OD(2 * (pc - 2))
                    emit_OD(2 * (pc - 2) + 1)
            for kc in range(max(0, n_kc - 4), n_kc):
                emit_OD(kc)
        else:
            st1 = min(3, n_kc - 1)
            qld = min(2, n_kc - 2)
            for kc in range(n_kc):
                emit_S(kc)
                if kc == 1:
                    flush_pending(0)
                if kc == st1:
                    flush_pending(1)
                if kc == qld and qt == 0 and bi + 1 < len(heads):
                    qtt_next = emit_qload(*heads[bi + 1])
                    obh_next = sb_o.tile([P, QL], F32, tag="obh")
                if kc >= PIPE:
                    emit_OD(kc - PIPE)
            for kc in range(max(0, n_kc - PIPE), n_kc):
                emit_OD(kc)

        def emit_norm(O_ps=O_ps, D_ps=D_ps, Obh=Obh, q0=q0):
            if "norm" in ABLATE:
                nc.vector.tensor_copy(Obh[:, q0:q0 + QT], O_ps[:])
            elif DEN_BCAST:
                den = sb_o.tile([P, QT], F32, tag="den")
                nc.vector.tensor_copy(den[:], D_ps[:, :])
                nc.vector.reciprocal(den[:], den[:])
                nc.vector.tensor_mul(Obh[:, q0:q0 + QT], O_ps[:], den[:])
            else:
                den = sb_o.tile([1, QT], F32, tag="den")
                nc.vector.tensor_copy(den[:], D_ps[:1, :])
                nc.vector.reciprocal(den[:], den[:])
                bc_sb = sb_o.tile([P, QT], F32, tag="bcsb")
                nc.gpsimd.partition_broadcast(bc_sb[:], den[:])
                nc.vector.tensor_mul(Obh[:, q0:q0 + QT], O_ps[:], bc_sb[:])

        pending.append((0, emit_norm))
        if qt == n_qt - 1:
            def emit_store(Obh=Obh, b=b, h=h):
                nc.scalar.dma_start(o_ap[b, h], Obh[:])
            pending.append((1, emit_store))
            qtt_cur, obh_cur = qtt_next, obh_next
    flush_pending(1)


def build_nc(B=2, QL=2048, KL=2048, HL=4, D=128, dt_mm=None, reps=1):
    nc = bacc.Bacc("TRN2", target_bir_lowering=False, debug=False,
                   num_devices=8)
    q = nc.dram_tensor("q", [B, HL, D, QL], F32, kind="ExternalInput")
    k = nc.dram_tensor("k", [B, D, KL], F32, kind="ExternalInput")
    v = nc.dram_tensor("v", [B, P, KL // P, P], F32, kind="ExternalInput")
    o = nc.dram_tensor("out", [B, HL, D, QL], F32, kind="ExternalOutput")
    from contextlib import ExitStack
    with tile.TileContext(nc) as tc:
        for _ in range(reps):
            with ExitStack() as ctx:
                emit_attention(nc, tc, ctx, q.ap(), k.ap(), v.ap(), o.ap(),
                               B, QL, KL, HL, D, dt_mm=dt_mm)
    nc.compile()
    return nc


def shard_inputs(q, k, v, n_cores=8):
    B, QL, H, D = q.shape
    KL = k.shape[1]
    HL = H // n_cores
    in_maps = []
    for c in range(n_cores):
        in_maps.append({
            # [B, HL, D, QL]
            "q": np.ascontiguousarray(
                q[:, :, HL * c:HL * (c + 1), :].transpose(0, 2, 3, 1)),
            # [B, D, KL]
            "k": np.ascontiguousarray(k[:, :, c, :].transpose(0, 2, 1)),
            # [B, P, KL/P, P]: partition = position within a 128-chunk
            "v": np.ascontiguousarray(
                v[:, :, c, :].reshape(B, KL // P, P, D).transpose(0, 2, 1, 3)),
        })
    return in_maps


_NC_CACHE = {}


def kernel(q: np.ndarray, k: np.ndarray, v: np.ndarray) -> np.ndarray:
    B, QL, H, D = q.shape
    KL, KVH = k.shape[1], k.shape[2]
    n_cores = 8
    HL = H // n_cores            # q-heads per core (4)
    assert KVH == n_cores and H == 32 and D == 128

    if "nc" not in _NC_CACHE:
        _NC_CACHE["nc"] = build_nc(B=B, QL=QL, KL=KL, HL=HL, D=D)
    nc = _NC_CACHE["nc"]

    q = np.asarray(q, dtype=np.float32)
    k = np.asarray(k, dtype=np.float32)
    v = np.asarray(v, dtype=np.float32)
    in_maps = shard_inputs(q, k, v, n_cores)
    res = run_bass_kernel_spmd(nc, in_maps, list(range(n_cores)))
    # device output is [B, HL, D, QL] per core -> [B, QL, HL, D], concat heads
    return np.concatenate(
        [r["out"].transpose(0, 3, 1, 2) for r in res.results], axis=2)
